# revision 23
# baseline (speedup 1.0000x reference)
"""Multi-head attention (B=4, S=2048, E=768, H=12, D=64, causal) on 8 trn2
NeuronCores.

Sharding: core c -> batch b = c//2, head-half g = c%2 (6 heads each).
Each core computes its 6 heads' attention plus the partial output
projection; the host sums the two half-head partials per batch.

On-device strategy (per core):
  - Host pre-transposes x[b] and the weight slices so every matmul
    contraction dim (e / d / k / e_h) lands on SBUF partitions; no
    on-device transposes.
  - QK projection emits qk^T [f, s]; V projection emits V [s, f] --
    exactly the operand orientations the attention matmuls need.
  - Scores are computed TRANSPOSED (S^T[k, q] = K^T Q) so the exp'd
    tiles E^T[k, q] feed the ctx matmul (ctx^T = V_aug^T E^T) directly.
  - A ones-column packed into V_aug makes the PE compute the softmax
    row-sums as ctx^T row 64 for free.
  - Causal masking: fully-masked tiles skipped; diagonal tiles get a
    -1e9 strict-lower-triangle added via a bf16 matmul into the same
    PSUM accumulation group.
  - Softmax normalization: rinv = 1/rowsum (DVE), broadcast across
    partitions with a K=1 matmul, multiplied in while leaving PSUM.
  - All big matmuls run in float32r (fp32 storage, 8-bit-mantissa
    matmul) at 1 cycle/row: 4x faster than fp32 matmul on trn2.
  - ctx matmuls are software-pipelined one step behind the scores
    matmuls so the in-order PE never stalls waiting for ACT's exp.
  - One PSUM pool with fixed tags (16 KB/partition exactly) is shared
    by all phases so work pipelines through buffer rotation.
"""
import sys, json, os

for _p in ("/opt/trn_rl_repo",):
    if _p not in sys.path and os.path.isdir(_p):
        sys.path.insert(0, _p)

import numpy as np
import concourse.bass as bass
import concourse.mybir as mybir
import concourse.tile as tile
from concourse.bass_utils import run_bass_kernel_spmd

B, S, E, H, D = 4, 2048, 768, 12, 64
HPC = H // 2          # heads per core = 6
FPC = HPC * D         # features per core per q/k/v = 384
EC = E // 128         # 6 contraction chunks for projections
SC = S // 128         # 16 s-chunks
QW = S // 512         # 4 q-windows
KC = S // 128         # 16 k-chunks
F32 = mybir.dt.float32
F32R = mybir.dt.float32r
BF16 = mybir.dt.bfloat16
EXP = mybir.ActivationFunctionType.Exp
NEG = -1.0e9


def round_f32r(a: np.ndarray) -> np.ndarray:
    """Round fp32 -> fp32r (8 explicit mantissa bits), RNE, as fp32 bits."""
    a = np.ascontiguousarray(a, dtype=np.float32)
    u = a.view(np.uint32).astype(np.uint64)
    u2 = (u + 0x3FFF + ((u >> 15) & 1)) & np.uint64(0xFFFF8000)
    return u2.astype(np.uint32).view(np.float32)


def _patch_multiwait(nc, max_waits=1):
    """This container's walrus rejects instructions with more than one sync
    wait. Split excess waits onto same-engine NOPs emitted immediately
    before the instruction (same-engine streams are order-preserving)."""
    raw = nc.to_json_bytes()
    m = json.loads(raw)
    for f in m["functions"]:
        for b in f["blocks"]:
            out = []
            for inst in b["instructions"]:
                si = inst.get("sync_info") or {}
                ws = si.get("on_wait") or []
                if len(ws) > max_waits:
                    eng = inst["engine"]
                    for i, w in enumerate(ws[:-max_waits]):
                        out.append({
                            "debug": inst.get("debug", 0), "engine": eng,
                            "ins": [], "name": inst["name"] + f"-mw{i}",
                            "opcode": "NoOp", "outs": [],
                            "sync_info": {"on_update": [], "on_wait": [w]},
                        })
                    si["on_wait"] = ws[-max_waits:]
                out.append(inst)
            b["instructions"] = out
    patched = json.dumps(m).encode()
    nc.to_json_bytes = lambda: patched
    return nc


def build_nc(repeat=1, with_bias=True):
    nc = bass.Bass()
    xT = nc.dram_tensor("xT", [E, S], F32R, kind="ExternalInput")
    wqkT = nc.dram_tensor("wqkT", [E, 2 * FPC], F32R, kind="ExternalInput")
    wvT = nc.dram_tensor("wvT", [E, FPC], F32R, kind="ExternalInput")
    woT = nc.dram_tensor("woT", [FPC, E], F32R, kind="ExternalInput")
    bqk = nc.dram_tensor("bqk", [128, 2 * FPC // 128], F32, kind="ExternalInput")
    bv = nc.dram_tensor("bv", [1, FPC], F32R, kind="ExternalInput")
    bo = nc.dram_tensor("bo", [1, E], F32R, kind="ExternalInput")
    tri = nc.dram_tensor("tri", [128, 128], BF16, kind="ExternalInput")
    ident = nc.dram_tensor("ident", [128, 128], BF16, kind="ExternalInput")
    ones = nc.dram_tensor("ones", [1, 128], F32R, kind="ExternalInput")
    y = nc.dram_tensor("y", [S, E], F32, kind="ExternalOutput")

    with tile.TileContext(nc) as tc, \
         nc.allow_low_precision(reason="f32r matmul pipeline by design"):
        with tc.tile_pool(name="persist", bufs=1) as P, \
             tc.tile_pool(name="ps", bufs=1, space="PSUM") as PS:
            # --- persistent tiles (bottom-of-stack, live whole kernel)
            qkT_sb = [P.tile([128, S], F32R, name=f"qkT{i}") for i in range(6)]
            V_sb = [P.tile([128, 65 * HPC], F32R, name=f"V{i}") for i in range(KC)]
            ctxT_sb = [P.tile([128, S], F32R, name=f"ctxT{i}") for i in range(3)]
            woT_sb = [P.tile([128, E], F32R, name=f"woT{i}") for i in range(3)]
            bqk_sb = P.tile([128, 6], F32, name="bqk_sb")
            bv_sb = P.tile([1, FPC], F32R, name="bv_sb")
            bo_sb = P.tile([1, E], F32R, name="bo_sb")
            tri_sb = P.tile([128, 128], BF16, name="tri_sb")
            id_sb = P.tile([128, 128], BF16, name="id_sb")
            on_sb = P.tile([1, 128], F32R, name="on_sb")

            def ps_tile(shape, tag, bufs):
                return PS.tile(shape, F32, name=tag, tag=tag, bufs=bufs)

            # ================= phase 1: projections =================
            with tc.tile_pool(name="inp", bufs=1) as PI:
                xT_sb = [PI.tile([128, S], F32R, name=f"xT{i}") for i in range(EC)]
                wqkT_sb = [PI.tile([128, 2 * FPC], F32R, name=f"wqkT{i}")
                           for i in range(EC)]
                wvT_sb = [PI.tile([128, FPC], F32R, name=f"wvT{i}")
                          for i in range(EC)]
                # DMA order: the qk chains consume (xT[ec] all windows,
                # wqkT[ec] col fo=0) in e-chunk order -- ship exactly that.
                nc.sync.dma_start(bqk_sb[:], bqk.ap())
                for i in range(EC):
                    nc.sync.dma_start(xT_sb[i][:, 0:1024],
                                      xT.ap()[128 * i:128 * (i + 1), 0:1024])
                    nc.sync.dma_start(xT_sb[i][:, 1024:S],
                                      xT.ap()[128 * i:128 * (i + 1), 1024:S])
                    nc.sync.dma_start(wqkT_sb[i][:, 0:128],
                                      wqkT.ap()[128 * i:128 * (i + 1), 0:128])
                for i in range(EC):
                    nc.sync.dma_start(
                        wqkT_sb[i][:, 128:2 * FPC],
                        wqkT.ap()[128 * i:128 * (i + 1), 128:2 * FPC])
                for i in range(EC):
                    nc.sync.dma_start(wvT_sb[i][:],
                                      wvT.ap()[128 * i:128 * (i + 1), :])
                nc.sync.dma_start(bv_sb[:], bv.ap())
                nc.sync.dma_start(tri_sb[:], tri.ap())
                nc.sync.dma_start(id_sb[:], ident.ap())
                nc.sync.dma_start(on_sb[:], ones.ap())
                for i in range(3):
                    nc.sync.dma_start(woT_sb[i][:],
                                      woT.ap()[128 * i:128 * (i + 1), :])
                nc.sync.dma_start(bo_sb[:], bo.ap())

                # qk-proj: 4 concurrent s-window chains per f-chunk so the
                # stationary wqkT block is loaded once per e-chunk (the PE
                # elides repeated self-loads) and consecutive matmuls hit
                # alternating PSUM banks.
                for fo in range(6):
                    pairs = [ps_tile([128, 1024], "pss_t", 2) for _ in range(2)]
                    for ecc in range(EC):
                        for sw in range(QW):
                            nc.tensor.matmul(
                                pairs[sw // 2][:, 512 * (sw % 2):
                                               512 * (sw % 2 + 1)],
                                wqkT_sb[ecc][:, 128 * fo:128 * (fo + 1)],
                                xT_sb[ecc][:, 512 * sw:512 * (sw + 1)],
                                start=(ecc == 0), stop=(ecc == EC - 1),
                                skip_group_check=True)
                    for p in range(2):
                        if with_bias:
                            nc.vector.tensor_scalar_add(
                                qkT_sb[fo][:, 1024 * p:1024 * (p + 1)],
                                pairs[p][:], bqk_sb[:, fo:fo + 1])
                        else:
                            nc.vector.tensor_copy(
                                qkT_sb[fo][:, 1024 * p:1024 * (p + 1)],
                                pairs[p][:])
                # V-proj: interleave s-chunk pairs across two PSUM banks
                for scp in range(SC // 2):
                    psvs = [ps_tile([128, FPC], "psc_t", 2) for _ in range(2)]
                    for ecc in range(EC):
                        for p in range(2):
                            sc = 2 * scp + p
                            nc.tensor.matmul(
                                psvs[p][:],
                                xT_sb[ecc][:, 128 * sc:128 * (sc + 1)],
                                wvT_sb[ecc][:],
                                start=(ecc == 0),
                                stop=(not with_bias and ecc == EC - 1),
                                skip_group_check=True)
                    for p in range(2):
                        sc = 2 * scp + p
                        if with_bias:
                            nc.tensor.matmul(psvs[p][:], on_sb[:, 0:128],
                                             bv_sb[:], start=False, stop=True,
                                             skip_group_check=True)
                        vv = V_sb[sc][:].rearrange("p (h x) -> p h x", x=65)
                        nc.vector.tensor_copy(
                            vv[:, :, 0:64],
                            psvs[p][:].rearrange("p (h x) -> p h x", x=64))
                        nc.gpsimd.memset(vv[:, :, 64:65].bitcast(F32), 1.0)

            # ================= phase 2+3: attention + out-proj ==============
            with tc.tile_pool(name="esb", bufs=6) as EP, \
                 tc.tile_pool(name="nrm", bufs=6) as NP, \
                 tc.tile_pool(name="osb", bufs=3) as OP:

                def emit_scores(hp, qw, u):
                    """Scores (pair of k-chunks) for both heads + exp on the
                    [128,1024] pair tile. Returns {hd: (pss, E)}."""
                    qT, kT = qkT_sb[hp], qkT_sb[3 + hp]
                    Es = {}
                    for hd in range(2):
                        Es[hd] = (ps_tile([128, 1024], "pss_t", 2),
                                  EP.tile([128, 1024], F32R, name="E_t"))
                    # strict row-group alternation (base 0,64,0,64) so the
                    # K=64 score matmul pairs run concurrently on the PE
                    for half in range(2):
                        ki = 2 * u + half
                        j = ki - 4 * qw
                        diag = j >= 0
                        for hd in range(2):
                            base = 64 * hd
                            pss = Es[hd][0]
                            nc.tensor.matmul(
                                pss[:, 512 * half:512 * (half + 1)],
                                kT[base:base + 64, 128 * ki:128 * (ki + 1)],
                                qT[base:base + 64, 512 * qw:512 * (qw + 1)],
                                start=True, stop=not diag,
                                skip_group_check=True)
                        if diag:
                            for hd in range(2):
                                pss = Es[hd][0]
                                nc.tensor.matmul(
                                    pss[:, 512 * half + 128 * j:
                                        512 * half + 128 * (j + 1)],
                                    id_sb[:], tri_sb[:],
                                    start=False, stop=True,
                                    skip_group_check=True)
                    for hd in range(2):
                        pss, Et = Es[hd]
                        j0 = 2 * u - 4 * qw
                        c0 = 128 * j0 if j0 > 0 else 0
                        c1 = 128 * (j0 + 1) if j0 + 1 > 0 else 0
                        if c0 == 0 and c1 == 0:
                            nc.scalar.activation(Et[:], pss[:], EXP, scale=0.125)
                        else:
                            nc.scalar.activation(Et[:, c0:512], pss[:, c0:512],
                                                 EXP, scale=0.125)
                            nc.scalar.activation(Et[:, 512 + c1:1024],
                                                 pss[:, 512 + c1:1024],
                                                 EXP, scale=0.125)
                    return Es

                def emit_ctx(hp, qw, u, Es, psc):
                    nki = 4 * qw + 4
                    for half in range(2):
                        ki = 2 * u + half
                        j = ki - 4 * qw
                        c = 128 * j if j > 0 else 0
                        for hd in range(2):
                            _, Et = Es[hd]
                            h = 2 * hp + hd
                            nc.tensor.matmul(
                                psc[hd][:, c:512],
                                V_sb[ki][:, 65 * h:65 * h + 65],
                                Et[:, 512 * half + c:512 * (half + 1)],
                                start=(ki == 0), stop=(ki == nki - 1),
                                skip_group_check=True)

                def emit_norm(hp, qw, psc):
                    """Copy ctx'+rowsum out of PSUM (freeing it), then
                    normalize into ctxT."""
                    for hd in range(2):
                        craw = NP.tile([65, 512], F32, name="craw_t")
                        nc.vector.tensor_copy(craw[:], psc[hd][:])
                        rinv = NP.tile([1, 512], F32R, name="rinv_t")
                        nc.vector.reciprocal(rinv[:], craw[64:65, :])
                        pb = ps_tile([64, 512], "pb_t", 1)
                        nc.tensor.matmul(pb[:], on_sb[:, 0:64], rinv[:],
                                         start=True, stop=True)
                        bc = NP.tile([64, 512], F32, name="bc_t")
                        nc.vector.tensor_copy(bc[:], pb[:])
                        nc.vector.tensor_mul(
                            ctxT_sb[hp][64 * hd:64 * (hd + 1),
                                        512 * qw:512 * (qw + 1)],
                            craw[0:64, :], bc[:])

                def emit_outproj(qw):
                    for sc in range(4 * qw, 4 * qw + 4):
                        osb = OP.tile([128, E], F32, name="osb_t")
                        # two f-window chains on two PSUM slots, c-outer so
                        # the ctxT stationary is loaded once per c
                        pos = {0: ps_tile([128, 512], "po_t", 1),
                               512: ps_tile([128, 512], "pb_t", 1)}
                        for c in range(3):
                            for f0, fn in ((0, 512), (512, 256)):
                                nc.tensor.matmul(
                                    pos[f0][:, 0:fn],
                                    ctxT_sb[c][:, 128 * sc:128 * (sc + 1)],
                                    woT_sb[c][:, f0:f0 + fn],
                                    start=(c == 0),
                                    stop=(not with_bias and c == 2),
                                    skip_group_check=True)
                        for f0, fn in ((0, 512), (512, 256)):
                            if with_bias:
                                nc.tensor.matmul(pos[f0][:, 0:fn],
                                                 on_sb[:, 0:128],
                                                 bo_sb[:, f0:f0 + fn],
                                                 start=False, stop=True,
                                                 skip_group_check=True)
                            nc.vector.tensor_copy(osb[:, f0:f0 + fn],
                                                  pos[f0][:, 0:fn])
                        nc.sync.dma_start(y.ap()[128 * sc:128 * (sc + 1), :],
                                          osb[:])

                def emit_attention():
                    # software pipeline: ctx trails scores by one step
                    # (depth 2 measured slower on HW: the 2-slot ctx
                    # accumulator rotation serializes across head pairs)
                    DEPTH = 1
                    pending = []   # [(hp, qw, u, Es, psc, last_u), ...]

                    def flush_one():
                        php, pqw, pu, pEs, ppsc, plast = pending.pop(0)
                        emit_ctx(php, pqw, pu, pEs, ppsc)
                        if pu == plast:
                            emit_norm(php, pqw, ppsc)
                            if php == 2:
                                emit_outproj(pqw)

                    for qw in range(QW):
                        for hp in range(3):
                            nu = (4 * qw + 4) // 2
                            psc = {hd: ps_tile([65, 512], "psc_t", 2)
                                   for hd in range(2)}
                            for u in range(nu):
                                Es = emit_scores(hp, qw, u)
                                if len(pending) >= DEPTH:
                                    flush_one()
                                pending.append((hp, qw, u, Es, psc, nu - 1))
                    while pending:
                        flush_one()

                if repeat == 1:
                    emit_attention()
                else:
                    with tc.For_i(0, repeat, 1):
                        emit_attention()

    return _patch_multiwait(nc)


_NC = {}


def _get_nc(with_bias=True):
    if with_bias not in _NC:
        _NC[with_bias] = build_nc(with_bias=with_bias)
    return _NC[with_bias]


def _prep_core_inputs(x, in_proj_w, in_proj_b, out_w, out_b):
    """Build the 8 per-core input dicts (host-side shard + transpose)."""
    import ml_dtypes
    tri_np = np.where(np.arange(128)[None, :] < np.arange(128)[:, None],
                      np.float32(NEG), np.float32(0.0))
    tri_bf = tri_np.astype(ml_dtypes.bfloat16)
    id_bf = np.eye(128, dtype=np.float32).astype(ml_dtypes.bfloat16)
    ones_np = round_f32r(np.ones((1, 128), np.float32))

    xT_by_b = [round_f32r(np.asarray(x[b]).T) for b in range(B)]

    in_maps = []
    for c in range(8):
        b = c // 2
        g = c % 2
        f0 = FPC * g
        Wq = np.asarray(in_proj_w[f0:f0 + FPC])
        Wk = np.asarray(in_proj_w[E + f0:E + f0 + FPC])
        Wv = np.asarray(in_proj_w[2 * E + f0:2 * E + f0 + FPC])
        bq = np.asarray(in_proj_b[f0:f0 + FPC])
        bk = np.asarray(in_proj_b[E + f0:E + f0 + FPC])
        bvv = np.asarray(in_proj_b[2 * E + f0:2 * E + f0 + FPC])
        Wo = np.asarray(out_w[:, f0:f0 + FPC])
        bqk_np = np.concatenate([bq, bk]).astype(np.float32).reshape(6, 128).T
        in_maps.append({
            "xT": xT_by_b[b],
            "wqkT": round_f32r(np.concatenate([Wq, Wk], axis=0).T),
            "wvT": round_f32r(Wv.T),
            "woT": round_f32r(Wo.T),
            "bqk": np.ascontiguousarray(bqk_np),
            "bv": round_f32r(bvv.reshape(1, FPC)),
            # out bias only on even cores so the host-side pair-sum is exact
            "bo": round_f32r(np.asarray(out_b).reshape(1, E)) if g == 0
                  else np.zeros((1, E), np.float32),
            "tri": tri_bf,
            "ident": id_bf,
            "ones": ones_np,
        })
    return in_maps


def kernel(x, in_proj_w, in_proj_b, out_w, out_b):
    zero_bias = (not np.any(np.asarray(in_proj_b))) and \
                (not np.any(np.asarray(out_b)))
    nc = _get_nc(with_bias=not zero_bias)
    in_maps = _prep_core_inputs(x, in_proj_w, in_proj_b, out_w, out_b)
    res = run_bass_kernel_spmd(nc, in_maps, core_ids=list(range(8)))
    out = np.empty((B, S, E), np.float32)
    for b in range(B):
        out[b] = res.results[2 * b]["y"] + res.results[2 * b + 1]["y"]
    return out
